# revision 13
# baseline (speedup 1.0000x reference)
"""Batched EKF predict+update (PositionFilter) Trainium2 Bass kernel.

Math per tracklet (B = 524288 independent tracklets, sharded over 8 cores):
    F       : fixed 8x8 constant-acceleration transition (dt = 0.1)
    x_pred  = F x
    P_pred  = F P F^T + Q
    S       = P_pred[:3,:3] + R          (3x3, symmetric)
    K       = P_pred[:,:3] inv(S)        (8x3)
    innov   = z - x_pred[:3]
    x_upd   = x_pred + K innov
    P_upd   = P_pred - K P_pred[:3,:]

Layout: batch-on-partition. Each chunk is [128 partitions, T tracklets]
with per-tracklet records along the free dim. All small-matrix algebra
becomes strided VectorE ops vectorized over 128*T tracklets; the 3x3
inverse is done via adjugate/cofactors. ScalarE takes the copies and
squares; DMA is all natural-layout with long contiguous runs.
"""

from contextlib import ExitStack

import numpy as np

from concourse import bass, mybir, tile

DT = 0.1
DT2 = 0.5 * DT * DT

F32 = mybir.dt.float32
MULT = mybir.AluOpType.mult
ADD = mybir.AluOpType.add

B_FULL = 524288
N_CORES = 8
B_LOCAL = B_FULL // N_CORES  # 65536


# --------------------------------------------------------------------------
# BIR wait-splitting pass.
#
# The walrus build in this container enforces a small per-instruction limit
# on sync-wait commands (1 for DMACopy / TensorScalarPtr, and a handful for
# the CTRL drain) while the Tile scheduler freely attaches several waits to
# one instruction (even the repo's own canonical tile example trips this).
# Rewrite the BIR before compiling: keep one wait on the instruction and
# hoist the rest onto standalone EventSemaphore wait instructions placed
# immediately before it on the same engine (the engine sequencer executes
# its instructions in block order, so the semantics are identical).
# --------------------------------------------------------------------------
_WAIT_SPLIT_INSTALLED = False


def _split_bir_waits(bir_json: bytes) -> bytes:
    import json

    bir = json.loads(bir_json)
    n_new = 0
    for fn in bir.get("functions", []):
        for blk in fn.get("blocks", []):
            insts = blk.get("instructions", [])
            out = []
            for inst in insts:
                si = inst.get("sync_info")
                ow = (si or {}).get("on_wait") or []
                if len(ow) > 1:
                    for i, w in enumerate(ow[:-1]):
                        out.append({
                            "debug": inst.get("debug", 0),
                            "engine": inst["engine"],
                            "ins": [],
                            "outs": [],
                            "name": f"{inst['name']}_hw{i}",
                            "opcode": "EventSemaphore",
                            "sync_info": {"on_update": [], "on_wait": [w]},
                        })
                        n_new += 1
                    si["on_wait"] = [ow[-1]]
                out.append(inst)
            blk["instructions"] = out
    return json.dumps(bir).encode()


def _install_wait_split():
    global _WAIT_SPLIT_INSTALLED
    if _WAIT_SPLIT_INSTALLED:
        return
    from concourse import bass_utils as _bu
    from concourse import bass2jax as _b2j

    _orig = _bu.compile_bir_kernel

    def _patched(bir_json, tmpdir, neff_name="file.neff"):
        return _orig(_split_bir_waits(bir_json), tmpdir, neff_name)

    _bu.compile_bir_kernel = _patched
    _b2j.compile_bir_kernel = _patched
    _WAIT_SPLIT_INSTALLED = True


def ekf_chunk(nc, pools, dram, c, T):
    """Emit instructions for one chunk of 128*T tracklets (chunk index c)."""
    io_pool, work_pool = pools
    x_d, P_d, z_d, Q_d, R_d, xu_d, Pu_d = dram
    CT = 128 * T
    v = nc.vector
    s = nc.scalar

    def dview(t, k):
        # [BL, k] dram AP -> [128, T, k] chunk view, T consecutive tracklets
        # per partition.
        return t[c * CT:(c + 1) * CT, :].rearrange("(p t) e -> p t e", p=128)

    # ---- load ----
    tP = io_pool.tile([128, T, 64], F32)
    tQ = io_pool.tile([128, T, 64], F32)
    tx = io_pool.tile([128, T, 8], F32)
    tz = io_pool.tile([128, T, 3], F32)
    tR = io_pool.tile([128, T, 9], F32)
    nc.gpsimd.dma_start(tP[:], dview(P_d, 64))
    nc.gpsimd.dma_start(tQ[:], dview(Q_d, 64))
    nc.gpsimd.dma_start(tx[:], dview(x_d, 8))
    nc.gpsimd.dma_start(tz[:], dview(z_d, 3))
    nc.gpsimd.dma_start(tR[:], dview(R_d, 9))

    P4 = tP.rearrange("p t (i j) -> p t i j", i=8)
    Q4 = tQ.rearrange("p t (i j) -> p t i j", i=8)

    # Wait-absorbing fences: the big DMA loads are split across many queues
    # (one semaphore each); STT/TT instructions have few sync-wait slots, so
    # take the multi-sem waits on cheap 1-element copies instead. Tile's
    # per-engine vector clock means later ops don't re-wait.
    tfence = work_pool.tile([128, 8], F32)
    v.tensor_copy(tfence[:, 0:1], tP[:, 0:1, 0])
    v.tensor_copy(tfence[:, 1:2], tQ[:, 0:1, 0])
    v.tensor_copy(tfence[:, 2:3], tx[:, 0:1, 0])
    v.tensor_copy(tfence[:, 3:4], tz[:, 0:1, 0])
    v.tensor_copy(tfence[:, 4:5], tR[:, 0:1, 0])

    # ---- x_pred = F x ----
    txp = work_pool.tile([128, T, 8], F32)
    v.scalar_tensor_tensor(txp[:, :, 0:5], tx[:, :, 3:8], DT, tx[:, :, 0:5],
                           MULT, ADD)
    v.scalar_tensor_tensor(txp[:, :, 0:2], tx[:, :, 6:8], DT2, txp[:, :, 0:2],
                           MULT, ADD)
    v.tensor_copy(txp[:, :, 5:8], tx[:, :, 5:8])

    # ---- A = F P (rows 0:5 computed; rows 5:8 copied from P) ----
    # STT is limited to 2 free dims, so operate on flat contiguous views:
    # rows 3:8 of the 8x8 record = elements 24:64.
    tA = work_pool.tile([128, T, 64], F32)
    v.scalar_tensor_tensor(tA[:, :, 0:40], tP[:, :, 24:64], DT,
                           tP[:, :, 0:40], MULT, ADD)
    v.scalar_tensor_tensor(tA[:, :, 0:16], tP[:, :, 48:64], DT2,
                           tA[:, :, 0:16], MULT, ADD)
    v.tensor_copy(tA[:, :, 40:64], tP[:, :, 40:64])

    # ---- B = A F^T (cols 0:5; all 8 rows at once via (t i) fused dim) ----
    tB = work_pool.tile([128, T, 8, 5], F32)
    Af = tA.rearrange("p t (i j) -> p (t i) j", i=8)
    Bf = tB.rearrange("p t i j -> p (t i) j")
    v.scalar_tensor_tensor(Bf[:], Af[:, :, 3:8], DT, Af[:, :, 0:5], MULT, ADD)
    v.scalar_tensor_tensor(Bf[:, :, 0:2], Af[:, :, 6:8], DT2, Bf[:, :, 0:2],
                           MULT, ADD)

    # ---- P_pred = [B | A-cols 5:8] + Q ----
    tPp = work_pool.tile([128, T, 64], F32)
    Pp4 = tPp.rearrange("p t (i j) -> p t i j", i=8)
    A4 = tA.rearrange("p t (i j) -> p t i j", i=8)
    v.tensor_add(Pp4[:, :, :, 0:5], tB[:], Q4[:, :, :, 0:5])
    v.tensor_add(Pp4[:, :, :, 5:8], A4[:, :, :, 5:8], Q4[:, :, :, 5:8])

    # ---- S = P_pred[:3,:3] + R (9 entries, row-major) ----
    tS = work_pool.tile([128, T, 9], F32)
    S3 = tS.rearrange("p t (i j) -> p t i j", i=3)
    R3 = tR.rearrange("p t (i j) -> p t i j", i=3)
    v.tensor_add(S3[:], Pp4[:, :, 0:3, 0:3], R3[:])

    # ---- cofactors of (symmetric) S; order [c00 c01 c02 c11 c12 c22] ----
    tsq = work_pool.tile([128, T, 3], F32)  # s5^2, s2^2, s1^2
    v.tensor_mul(tsq[:, :, 0:1], tS[:, :, 5:6], tS[:, :, 5:6])
    v.tensor_mul(tsq[:, :, 1:2], tS[:, :, 2:3], tS[:, :, 2:3])
    v.tensor_mul(tsq[:, :, 2:3], tS[:, :, 1:2], tS[:, :, 1:2])

    tcof = work_pool.tile([128, T, 6], F32)
    tta = work_pool.tile([128, T, 1], F32)

    def sl(t, i):
        return t[:, :, i:i + 1]

    # c00 = s4 s8 - s5^2
    v.tensor_mul(sl(tcof, 0), sl(tS, 4), sl(tS, 8))
    v.tensor_sub(sl(tcof, 0), sl(tcof, 0), sl(tsq, 0))
    # c01 = s2 s5 - s1 s8
    v.tensor_mul(sl(tcof, 1), sl(tS, 2), sl(tS, 5))
    v.tensor_mul(tta[:], sl(tS, 1), sl(tS, 8))
    v.tensor_sub(sl(tcof, 1), sl(tcof, 1), tta[:])
    # c02 = s1 s5 - s2 s4
    v.tensor_mul(sl(tcof, 2), sl(tS, 1), sl(tS, 5))
    v.tensor_mul(tta[:], sl(tS, 2), sl(tS, 4))
    v.tensor_sub(sl(tcof, 2), sl(tcof, 2), tta[:])
    # c11 = s0 s8 - s2^2
    v.tensor_mul(sl(tcof, 3), sl(tS, 0), sl(tS, 8))
    v.tensor_sub(sl(tcof, 3), sl(tcof, 3), sl(tsq, 1))
    # c12 = s1 s2 - s0 s5
    v.tensor_mul(sl(tcof, 4), sl(tS, 1), sl(tS, 2))
    v.tensor_mul(tta[:], sl(tS, 0), sl(tS, 5))
    v.tensor_sub(sl(tcof, 4), sl(tcof, 4), tta[:])
    # c22 = s0 s4 - s1^2
    v.tensor_mul(sl(tcof, 5), sl(tS, 0), sl(tS, 4))
    v.tensor_sub(sl(tcof, 5), sl(tcof, 5), sl(tsq, 2))

    # ---- det = s0 c00 + s1 c01 + s2 c02 ; rdet = 1/det ----
    tdm = work_pool.tile([128, T, 3], F32)
    v.tensor_mul(tdm[:], tS[:, :, 0:3], tcof[:, :, 0:3])
    tdet = work_pool.tile([128, T, 1], F32)
    v.tensor_add(tdet[:], sl(tdm, 0), sl(tdm, 1))
    v.tensor_add(tdet[:], tdet[:], sl(tdm, 2))
    trdet = work_pool.tile([128, T, 1], F32)
    v.reciprocal(trdet[:], tdet[:])

    # ---- invS (full 9, row-major) = adj(S) * rdet ----
    tinv = work_pool.tile([128, T, 9], F32)
    rb3 = trdet.broadcast_to([128, T, 3])
    v.tensor_mul(tinv[:, :, 0:3], tcof[:, :, 0:3], rb3)
    v.tensor_mul(tinv[:, :, 4:6], tcof[:, :, 3:5],
                 trdet.broadcast_to([128, T, 2]))
    v.tensor_mul(tinv[:, :, 8:9], tcof[:, :, 5:6], trdet[:])
    v.tensor_copy(tinv[:, :, 3:4], tinv[:, :, 1:2])
    v.tensor_copy(tinv[:, :, 6:7], tinv[:, :, 2:3])
    v.tensor_copy(tinv[:, :, 7:8], tinv[:, :, 5:6])

    # ---- K = P_pred[:,:3] invS  ([8,3], 3-term dots) ----
    tK = work_pool.tile([128, T, 8, 3], F32)
    tKt = work_pool.tile([128, T, 8, 3], F32)
    inv3 = tinv.rearrange("p t (m j) -> p t m j", m=3)
    for m in range(3):
        a = Pp4[:, :, :, m:m + 1].broadcast_to([128, T, 8, 3])
        b = inv3[:, :, m:m + 1, :].broadcast_to([128, T, 8, 3])
        if m == 0:
            v.tensor_mul(tK[:], a, b)
        else:
            v.tensor_mul(tKt[:], a, b)
            v.tensor_add(tK[:], tK[:], tKt[:])

    # ---- innov = z - x_pred[:3] ; Kv = K innov ; x_upd ----
    tinn = work_pool.tile([128, T, 3], F32)
    v.tensor_sub(tinn[:], tz[:], txp[:, :, 0:3])
    tKv = work_pool.tile([128, T, 8], F32)
    tKvt = work_pool.tile([128, T, 8], F32)
    for m in range(3):
        a = tK[:, :, :, m]
        b = tinn[:, :, m:m + 1].broadcast_to([128, T, 8])
        if m == 0:
            v.tensor_mul(tKv[:], a, b)
        else:
            v.tensor_mul(tKvt[:], a, b)
            v.tensor_add(tKv[:], tKv[:], tKvt[:])
    txu = io_pool.tile([128, T, 8], F32)
    v.tensor_add(txu[:], txp[:], tKv[:])

    # ---- M = K P_pred[:3,:] (symmetric: top 3x8 block + lower-right 5x5) --
    tM1 = work_pool.tile([128, T, 3, 8], F32)
    tM1t = work_pool.tile([128, T, 3, 8], F32)
    for m in range(3):
        a = tK[:, :, 0:3, m:m + 1].broadcast_to([128, T, 3, 8])
        b = Pp4[:, :, m:m + 1, :].broadcast_to([128, T, 3, 8])
        if m == 0:
            v.tensor_mul(tM1[:], a, b)
        else:
            v.tensor_mul(tM1t[:], a, b)
            v.tensor_add(tM1[:], tM1[:], tM1t[:])
    tM2 = work_pool.tile([128, T, 5, 5], F32)
    tM2t = work_pool.tile([128, T, 5, 5], F32)
    for m in range(3):
        a = tK[:, :, 3:8, m:m + 1].broadcast_to([128, T, 5, 5])
        b = Pp4[:, :, m:m + 1, 3:8].broadcast_to([128, T, 5, 5])
        if m == 0:
            v.tensor_mul(tM2[:], a, b)
        else:
            v.tensor_mul(tM2t[:], a, b)
            v.tensor_add(tM2[:], tM2[:], tM2t[:])

    # ---- P_upd = P_pred - M (lower-left read from M1 transposed) ----
    tPu = io_pool.tile([128, T, 64], F32)
    Pu4 = tPu.rearrange("p t (i j) -> p t i j", i=8)
    v.tensor_sub(Pu4[:, :, 0:3, :], Pp4[:, :, 0:3, :], tM1[:])
    v.tensor_sub(Pu4[:, :, 3:8, 3:8], Pp4[:, :, 3:8, 3:8], tM2[:])
    v.tensor_sub(Pu4[:, :, 3:8, 0:3], Pp4[:, :, 3:8, 0:3],
                 tM1[:, :, :, 3:8].transpose([0, 1, 3, 2]))

    # ---- store ----
    nc.gpsimd.dma_start(dview(xu_d, 8), txu[:])
    nc.gpsimd.dma_start(dview(Pu_d, 64), tPu[:])
    return tPu, tfence


def ekf_body(ctx, tc, outs, ins, T):
    """Kernel body on DRAM APs. ins = (x,P,z,Q,R) 2-D; outs = (x_upd,P_upd)."""
    nc = tc.nc
    x_d, P_d, z_d, Q_d, R_d = ins
    xu_d, Pu_d = outs
    bl = x_d.shape[0]
    assert bl % 128 == 0 and (bl // 128) % T == 0
    nchunks = bl // (128 * T)
    io_pool = ctx.enter_context(tc.tile_pool(name="io", bufs=2))
    work_pool = ctx.enter_context(tc.tile_pool(name="work", bufs=1))
    dram = (x_d, P_d, z_d, Q_d, R_d, xu_d, Pu_d)
    for c in range(nchunks):
        ekf_chunk(nc, (io_pool, work_pool), dram, c, T)


def build_module(bl, T):
    """Build the Bass module for a per-core shard of bl tracklets."""
    _install_wait_split()
    nc = bass.Bass()
    x_d = nc.declare_dram_parameter("x", [bl, 8], F32, isOutput=False)
    P_d = nc.declare_dram_parameter("P", [bl, 64], F32, isOutput=False)
    z_d = nc.declare_dram_parameter("z", [bl, 3], F32, isOutput=False)
    Q_d = nc.declare_dram_parameter("Q", [bl, 64], F32, isOutput=False)
    R_d = nc.declare_dram_parameter("R", [bl, 9], F32, isOutput=False)
    xu_d = nc.declare_dram_parameter("x_upd", [bl, 8], F32, isOutput=True)
    Pu_d = nc.declare_dram_parameter("P_upd", [bl, 64], F32, isOutput=True)
    with tile.TileContext(nc) as tc:
        with ExitStack() as ctx:
            ekf_body(ctx, tc, (xu_d[:], Pu_d[:]),
                     (x_d[:], P_d[:], z_d[:], Q_d[:], R_d[:]), T)
    return nc


_CACHE = {}


def kernel(x, P, z, Q, R, _T=64, _trace=False):
    """Full-input entry point: shards B across 8 cores, returns full outputs."""
    from concourse.bass_utils import run_bass_kernel_spmd

    b = x.shape[0]
    bl = b // N_CORES
    key = (bl, _T)
    if key not in _CACHE:
        _CACHE[key] = build_module(bl, _T)
    nc = _CACHE[key]

    xf = np.ascontiguousarray(np.asarray(x, np.float32).reshape(b, 8))
    Pf = np.ascontiguousarray(np.asarray(P, np.float32).reshape(b, 64))
    zf = np.ascontiguousarray(np.asarray(z, np.float32).reshape(b, 3))
    Qf = np.ascontiguousarray(np.asarray(Q, np.float32).reshape(b, 64))
    Rf = np.ascontiguousarray(np.asarray(R, np.float32).reshape(b, 9))

    in_maps = []
    for i in range(N_CORES):
        sel = slice(i * bl, (i + 1) * bl)
        in_maps.append({"x": xf[sel], "P": Pf[sel], "z": zf[sel],
                        "Q": Qf[sel], "R": Rf[sel]})

    res = run_bass_kernel_spmd(nc, in_maps, list(range(N_CORES)),
                               trace=_trace)
    x_upd = np.concatenate([r["x_upd"] for r in res.results], 0)
    P_upd = np.concatenate([r["P_upd"] for r in res.results], 0)
    out = (x_upd.reshape(b, 8, 1), P_upd.reshape(b, 8, 8))
    if _trace:
        return out, res
    return out
